# revision 16
# baseline (speedup 1.0000x reference)
"""Trainium2 Bass kernel for BitNet multi-head attention (nn_MultiHeadAttention_62294205661880).

Sharding: 8 cores = 2 batches x 4 head-groups (4 heads each).  Each core
computes qkv projection, RoPE, causal attention and a column-parallel slice
of the output projection for its (batch, head-group); the host sums the 4
partial out-projections per batch (the tensor-parallel all-reduce done
host-side, since the contract gathers to host anyway).

BitNet quantization is folded on the host: weights are uploaded as exact
ternary {-1,0,+1} bf16 matrices; scale_qkv^2/sqrt(dh) is folded into the
softmax exp() scale and scale_qkv*scale_out into a final host-side scalar.

Device layout trick: everything is computed transposed.  Q_T/K_T come out of
the projection as [dh, S]; scores are computed as s_T[k, q]; the softmax
denominator sums over the partition (key) dim via an all-ones stationary
matmul (which also replicates the sums across partitions for free); AV
produces out_T[dh, q] which feeds the output projection directly.  No
on-device transposes at all.  Softmax skips the max-subtraction: scores are
bounded (~+-2) because the BitNet weight scale is tiny, so exp() is safe.
"""

import sys
import types

import numpy as np
import ml_dtypes

import concourse.bass as bass
import concourse.mybir as mybir
import concourse.tile as tile
from concourse import bacc
from concourse.bass_utils import run_bass_kernel_spmd

D_MODEL = 2048
N_HEADS = 16
D_HEAD = 128
SEQ = 2048
BATCH = 2
ROPE_BASE = 10000.0

N_CORES = 8
HPC = 4  # heads per core
R_LOCAL = HPC * D_HEAD  # 512 local q (or k, or v) rows per core
MO = D_MODEL // 128  # 16 contraction blocks
NKI = SEQ // 128  # 16 key blocks
NQC = SEQ // 512  # 4 query chunks of 512
NSB = SEQ // 128  # 16 seq blocks (v / proj)

BF16 = mybir.dt.bfloat16
F32 = mybir.dt.float32
NPBF16 = ml_dtypes.bfloat16

LAST_RESULT = None  # BassKernelResults of the most recent run (for test.py)
_PROG_CACHE = {}
PROFILE = False  # test.py sets True to capture an NTFF profile / HW exec time


def _enable_profiling() -> bool:
    """Install the axon NTFF profile hook glue if the image lacks
    ``antenv.axon_hooks`` (boot degrades silently without it), and skip
    the artifact upload (no bucket access in this container)."""
    try:
        from antenv.axon_hooks import get_axon_ntff_profile_hook  # noqa: F401

        ok = get_axon_ntff_profile_hook() is not None
    except ImportError:
        ok = False
        import antenv

        mod = types.ModuleType("antenv.axon_hooks")
        mod._hook = None
        mod.set_axon_ntff_profile_hook = lambda h: setattr(mod, "_hook", h)
        mod.get_axon_ntff_profile_hook = lambda: mod._hook
        sys.modules["antenv.axon_hooks"] = mod
        antenv.axon_hooks = mod
        try:
            from trn_agent_boot.trn_boot import _ntff_profile_via_ctypes

            hook = _ntff_profile_via_ctypes("/opt/axon/libaxon_pjrt.so")
            if hook is not None:
                mod._hook = hook
                ok = True
        except Exception as e:  # profiling is best-effort
            print(f"ntff profile hook install failed: {e}", file=sys.stderr)
    if ok:
        import concourse.bass_utils as _bu

        _bu.upload_artifacts = lambda tmpdir: tmpdir
    return ok


def _build_program(causal: bool, exp_scale: float) -> bass.Bass:
    nc = bacc.Bacc(None)
    S = SEQ

    xT_d = nc.dram_tensor("xT", [D_MODEL, S], BF16, kind="ExternalInput")
    wqT_d = nc.dram_tensor("wqT", [D_MODEL, R_LOCAL], BF16, kind="ExternalInput")
    wkT_d = nc.dram_tensor("wkT", [D_MODEL, R_LOCAL], BF16, kind="ExternalInput")
    wvT_d = nc.dram_tensor("wvT", [D_MODEL, R_LOCAL], BF16, kind="ExternalInput")
    woT_d = nc.dram_tensor("woT", [R_LOCAL, D_MODEL], BF16, kind="ExternalInput")
    # cos rows 0:64, sin rows 64:128
    cs_d = nc.dram_tensor("cossinT", [128, S], BF16, kind="ExternalInput")
    # swapped: sin rows 0:64, cos rows 64:128 (keeps TensorTensor base partitions equal)
    sc_d = nc.dram_tensor("sincosT", [128, S], BF16, kind="ExternalInput")
    if causal:
        # 16 transposed diagonal 128x128 mask blocks, side by side
        maskd_d = nc.dram_tensor("maskd", [128, S], BF16, kind="ExternalInput")
    else:
        maskf_d = nc.dram_tensor("maskf", [S, S], BF16, kind="ExternalInput")
    out_d = nc.dram_tensor("out", [S, D_MODEL], BF16, kind="ExternalOutput")

    xT_v = xT_d[:].rearrange("(mo p) s -> p mo s", p=128)
    wqT_v = wqT_d[:].rearrange("(mo p) r -> p mo r", p=128)
    wkT_v = wkT_d[:].rearrange("(mo p) r -> p mo r", p=128)
    wvT_v = wvT_d[:].rearrange("(mo p) r -> p mo r", p=128)
    woT_v = woT_d[:].rearrange("(h p) o -> p h o", p=128)
    if not causal:
        maskf_v = maskf_d[:].rearrange("(ko p) q -> p ko q", p=128)

    with tile.TileContext(nc) as tc:
        with tc.tile_pool(name="pers", bufs=1) as pers:
            # ---- persistent SBUF tensors (live across both phases) ----
            q_rot = pers.tile([128, HPC, S], BF16, tag="qrot")
            k_rot = pers.tile([128, HPC, S], BF16, tag="krot")
            v_sb = pers.tile([128, NKI, R_LOCAL], BF16, tag="vsb")
            aoT = pers.tile([128, HPC, S], BF16, tag="aoT")
            ones_t = pers.tile([128, 128], BF16, tag="ones")
            warm = pers.tile([128, 1], BF16, tag="warm")
            if causal:
                maskd = pers.tile([128, S], BF16, tag="maskd")
                nc.sync.dma_start(out=maskd[:, :], in_=maskd_d[:, :])
            nc.vector.memset(ones_t[:, :], 1.0)
            # load the exp table set first so no ACT table switch happens
            # mid-kernel (Copy lives in every set).
            nc.scalar.activation(
                warm[:, :], ones_t[:, 0:1], mybir.ActivationFunctionType.Exp
            )

            # ================= phase A: QKV projection + RoPE =================
            with (
                tc.tile_pool(name="xtp", bufs=1) as xtp,
                tc.tile_pool(name="wp", bufs=1) as wp,
                tc.tile_pool(name="raw", bufs=2) as rawp,
                tc.tile_pool(name="tmp", bufs=2) as tmpp,
                tc.tile_pool(name="psA", bufs=2, space="PSUM") as psA,
            ):
                xt = xtp.tile([128, MO, S], BF16, tag="xt")
                wq = wp.tile([128, MO, R_LOCAL], BF16, tag="wq")
                wk = wp.tile([128, MO, R_LOCAL], BF16, tag="wk")
                wv = wp.tile([128, MO, R_LOCAL], BF16, tag="wv")
                cs_t = wp.tile([128, S], BF16, tag="cs")
                sc_t = wp.tile([128, S], BF16, tag="sc")

                nc.sync.dma_start(out=cs_t[:, :], in_=cs_d[:, :])
                nc.sync.dma_start(out=sc_t[:, :], in_=sc_d[:, :])
                for mo in range(MO):
                    nc.sync.dma_start(out=wq[:, mo, :], in_=wqT_v[:, mo, :])
                    nc.sync.dma_start(out=wk[:, mo, :], in_=wkT_v[:, mo, :])
                    if mo < 4:
                        nc.sync.dma_start(
                            out=xt[:, mo, 0:1024], in_=xT_v[:, mo, 0:1024]
                        )
                        nc.sync.dma_start(
                            out=xt[:, mo, 1024:2048], in_=xT_v[:, mo, 1024:2048]
                        )
                    else:
                        nc.sync.dma_start(out=xt[:, mo, :], in_=xT_v[:, mo, :])
                for mo in range(MO):
                    nc.sync.dma_start(out=wv[:, mo, :], in_=wvT_v[:, mo, :])

                def rope(dst, raw):
                    """NeoX rotary: rows 0:64 = t*c - b*s ; rows 64:128 = t*s + b*c."""
                    ta = tmpp.tile([64, S], BF16, tag="tmp")
                    tb = tmpp.tile([64, S], BF16, tag="tmp")
                    nc.vector.tensor_mul(ta[:, :], raw[0:64, :], cs_t[0:64, :])
                    nc.vector.tensor_mul(tb[:, :], raw[64:128, :], cs_t[64:128, :])
                    nc.vector.tensor_sub(dst[0:64, :], ta[:, :], tb[:, :])
                    tc2 = tmpp.tile([64, S], BF16, tag="tmp")
                    td = tmpp.tile([64, S], BF16, tag="tmp")
                    nc.vector.tensor_mul(tc2[:, :], raw[0:64, :], sc_t[0:64, :])
                    nc.vector.tensor_mul(td[:, :], raw[64:128, :], sc_t[64:128, :])
                    nc.vector.tensor_add(dst[64:128, :], tc2[:, :], td[:, :])

                # head 0 q/k with the m-loop OUTER so the matmuls consume
                # xt m-blocks as the DMAs land (startup overlap).
                qp0 = psA.tile([128, S], F32, tag="psA")
                kp0 = psA.tile([128, S], F32, tag="psA")
                for m in range(MO):
                    for c4 in range(4):
                        nc.tensor.matmul(
                            qp0[:, c4 * 512 : (c4 + 1) * 512],
                            wq[:, m, 0:128],
                            xt[:, m, c4 * 512 : (c4 + 1) * 512],
                            start=(m == 0),
                            stop=(m == MO - 1),
                        )
                        nc.tensor.matmul(
                            kp0[:, c4 * 512 : (c4 + 1) * 512],
                            wk[:, m, 0:128],
                            xt[:, m, c4 * 512 : (c4 + 1) * 512],
                            start=(m == 0),
                            stop=(m == MO - 1),
                        )
                q_raw = rawp.tile([128, S], BF16, tag="raw")
                nc.scalar.copy(q_raw[:, :], qp0[:, :])
                rope(q_rot[:, 0, :], q_raw)
                k_raw = rawp.tile([128, S], BF16, tag="raw")
                nc.scalar.copy(k_raw[:, :], kp0[:, :])
                rope(k_rot[:, 0, :], k_raw)

                def project(dst_raw, w_sb, h):
                    """q/k head projection -> bf16 raw [128, S] (xt resident)."""
                    ps = psA.tile([128, S], F32, tag="psA")
                    for c4 in range(4):
                        for m in range(MO):
                            nc.tensor.matmul(
                                ps[:, c4 * 512 : (c4 + 1) * 512],
                                w_sb[:, m, h * 128 : (h + 1) * 128],
                                xt[:, m, c4 * 512 : (c4 + 1) * 512],
                                start=(m == 0),
                                stop=(m == MO - 1),
                            )
                    nc.scalar.copy(dst_raw[:, :], ps[:, :])

                for h in range(1, HPC):
                    q_raw = rawp.tile([128, S], BF16, tag="raw")
                    project(q_raw, wq, h)
                    rope(q_rot[:, h, :], q_raw)
                    k_raw = rawp.tile([128, S], BF16, tag="raw")
                    project(k_raw, wk, h)
                    rope(k_rot[:, h, :], k_raw)

                # V projection (natural layout [s, r]); 4 seq blocks per psum
                for sb4 in range(NSB // 4):
                    ps = psA.tile([128, S], F32, tag="psA")
                    for part in range(4):
                        sb = sb4 * 4 + part
                        for m in range(MO):
                            nc.tensor.matmul(
                                ps[:, part * 512 : part * 512 + 512],
                                xt[:, m, sb * 128 : (sb + 1) * 128],
                                wv[:, m, :],
                                start=(m == 0),
                                stop=(m == MO - 1),
                            )
                    nc.scalar.copy(v_sb[:, sb4 * 4 : sb4 * 4 + 4, :], ps[:, :])

            # ================= phase B: attention + out-projection =============
            with (
                tc.tile_pool(name="wop", bufs=1) as wop,
                tc.tile_pool(name="pp", bufs=8) as ppp,
                tc.tile_pool(name="rcp", bufs=3) as rcp,
                tc.tile_pool(name="osb", bufs=4) as osbp,
                tc.tile_pool(name="mblk", bufs=4) as mblkp,
                tc.tile_pool(name="sp", bufs=4, space="PSUM") as spp,
                tc.tile_pool(name="acc", bufs=2, space="PSUM") as accp,
            ):
                wo = wop.tile([128, HPC, D_MODEL], BF16, tag="wo")
                for oc in range(D_MODEL // 512):
                    nc.sync.dma_start(
                        out=wo[:, :, oc * 512 : (oc + 1) * 512],
                        in_=woT_v[:, :, oc * 512 : (oc + 1) * 512],
                    )

                evict_flip = [0]

                for qc in range(NQC):
                    q_lo = qc * 512
                    nki_here = (4 * qc + 4) if causal else NKI
                    for h in range(HPC):
                        sav = accp.tile([128, 1024], F32, tag="acc")
                        sums = sav[:, 0:512]
                        avp = sav[:, 512:1024]
                        for ki in range(nki_here):
                            diag = causal and ki >= 4 * qc
                            q0 = 128 * (ki - 4 * qc) if diag else 0
                            spb = spp.tile([128, 512], F32, tag="sp")
                            pp = ppp.tile([128, 512], BF16, tag="pp")
                            nc.tensor.matmul(
                                spb[:, q0:512],
                                k_rot[:, h, ki * 128 : (ki + 1) * 128],
                                q_rot[:, h, q_lo + q0 : q_lo + 512],
                                start=True,
                                stop=True,
                            )
                            nc.scalar.activation(
                                pp[:, q0:512],
                                spb[:, q0:512],
                                mybir.ActivationFunctionType.Exp,
                                scale=float(exp_scale),
                            )
                            if causal:
                                if diag:
                                    nc.vector.tensor_mul(
                                        pp[:, q0 : q0 + 128],
                                        pp[:, q0 : q0 + 128],
                                        maskd[:, ki * 128 : (ki + 1) * 128],
                                    )
                            else:
                                mb = mblkp.tile([128, 512], BF16, tag="mblk")
                                nc.sync.dma_start(
                                    out=mb[:, :],
                                    in_=maskf_v[:, ki, q_lo : q_lo + 512],
                                )
                                nc.vector.tensor_mul(
                                    pp[:, 0:512], pp[:, 0:512], mb[:, :]
                                )
                            nc.tensor.matmul(
                                sums[:, q0:512],
                                ones_t[:, :],
                                pp[:, q0:512],
                                start=(ki == 0),
                                stop=(ki == nki_here - 1),
                            )
                            nc.tensor.matmul(
                                avp[:, q0:512],
                                v_sb[:, ki, h * 128 : (h + 1) * 128],
                                pp[:, q0:512],
                                start=(ki == 0),
                                stop=(ki == nki_here - 1),
                            )
                        rc = rcp.tile([128, 512], F32, tag="rc")
                        nc.vector.reciprocal_approx_fast(rc[:, :], sums[:, :])
                        nc.vector.tensor_mul(
                            aoT[:, h, q_lo : q_lo + 512], avp[:, :], rc[:, :]
                        )

                    # out-projection for this query chunk (4 seq blocks);
                    # h outer over oc pairs so each aoT stationary load
                    # serves two matmuls
                    for sb in range(4 * qc, 4 * qc + 4):
                        for oc2 in range(2):
                            op2 = accp.tile([128, 1024], F32, tag="acc")
                            for h in range(HPC):
                                lhsT = aoT[:, h, sb * 128 : (sb + 1) * 128]
                                nc.tensor.matmul(
                                    op2[:, 0:512],
                                    lhsT,
                                    wo[:, h, (2 * oc2) * 512 : (2 * oc2 + 1) * 512],
                                    start=(h == 0),
                                    stop=(h == HPC - 1),
                                )
                                nc.tensor.matmul(
                                    op2[:, 512:1024],
                                    lhsT,
                                    wo[:, h, (2 * oc2 + 1) * 512 : (2 * oc2 + 2) * 512],
                                    start=(h == 0),
                                    stop=(h == HPC - 1),
                                )
                            ob = osbp.tile([128, 1024], BF16, tag="osb")
                            # final chunk: halve evict granularity so the tail
                            # drain (evict -> DMA) pipelines across engines
                            if qc == NQC - 1:
                                nc.scalar.copy(ob[:, 0:512], op2[:, 0:512])
                                nc.vector.tensor_copy(ob[:, 512:1024], op2[:, 512:1024])
                                nc.sync.dma_start(
                                    out=out_d[
                                        sb * 128 : (sb + 1) * 128,
                                        oc2 * 1024 : oc2 * 1024 + 512,
                                    ],
                                    in_=ob[:, 0:512],
                                )
                                nc.sync.dma_start(
                                    out=out_d[
                                        sb * 128 : (sb + 1) * 128,
                                        oc2 * 1024 + 512 : (oc2 + 1) * 1024,
                                    ],
                                    in_=ob[:, 512:1024],
                                )
                            else:
                                if evict_flip[0] % 2 == 0:
                                    nc.scalar.copy(ob[:, :], op2[:, :])
                                else:
                                    nc.vector.tensor_copy(ob[:, :], op2[:, :])
                                evict_flip[0] += 1
                                nc.sync.dma_start(
                                    out=out_d[
                                        sb * 128 : (sb + 1) * 128,
                                        oc2 * 1024 : (oc2 + 1) * 1024,
                                    ],
                                    in_=ob[:, :],
                                )

    nc.finalize()
    return nc


def _bit_quantize_ternary(w: np.ndarray):
    """Returns (ternary {-1,0,1} float32 matrix, scale) matching the reference."""
    scale = np.maximum(np.mean(np.abs(w.astype(np.float32))), np.float32(1e-5))
    t = np.clip(np.round(w.astype(np.float32) / scale), -1.0, 1.0).astype(np.float32)
    return t, float(scale)


def _host_tables():
    """cos/sin stacked [128, S]: rows 0:64 cos, rows 64:128 sin."""
    inv_freq = 1.0 / (ROPE_BASE ** (np.arange(0, D_HEAD, 2, dtype=np.float32) / D_HEAD))
    pos = np.arange(SEQ, dtype=np.float32)
    ang = pos[:, None] * inv_freq[None, :]  # [S, 64]
    cs = np.empty((128, SEQ), dtype=NPBF16)
    cs[0:64] = np.ascontiguousarray(np.cos(ang).T).astype(NPBF16)
    cs[64:128] = np.ascontiguousarray(np.sin(ang).T).astype(NPBF16)
    sc = np.empty((128, SEQ), dtype=NPBF16)
    sc[0:64] = cs[64:128]
    sc[64:128] = cs[0:64]
    return cs, sc


def kernel(x, w_qkv, w_out, mask):
    global LAST_RESULT
    x = np.asarray(x, dtype=np.float32)
    w_qkv = np.asarray(w_qkv, dtype=np.float32)
    w_out = np.asarray(w_out, dtype=np.float32)
    mask = np.asarray(mask)

    tq, sq = _bit_quantize_ternary(w_qkv)
    to, so = _bit_quantize_ternary(w_out)
    exp_scale = (sq * sq) / float(np.sqrt(D_HEAD))
    c2 = np.float32(sq * so)

    m2 = (mask.reshape(SEQ, SEQ) != 0).astype(np.float32)
    causal = bool(np.array_equal(m2, np.tril(np.ones((SEQ, SEQ), np.float32))))

    cs, sc = _host_tables()
    if causal:
        maskd = np.empty((128, SEQ), dtype=NPBF16)
        for ki in range(NKI):
            blk = m2[ki * 128 : (ki + 1) * 128, ki * 128 : (ki + 1) * 128]  # [q, k]
            maskd[:, ki * 128 : (ki + 1) * 128] = np.ascontiguousarray(blk.T).astype(
                NPBF16
            )
    else:
        maskf = np.ascontiguousarray(m2.T).astype(NPBF16)  # [kk, qq]

    key = (causal, float(exp_scale))
    if key not in _PROG_CACHE:
        _PROG_CACHE[key] = _build_program(causal, float(exp_scale))
    nc = _PROG_CACHE[key]

    in_maps = []
    for c in range(N_CORES):
        b, g = divmod(c, 4)
        rows = slice(R_LOCAL * g, R_LOCAL * (g + 1))
        im = {
            "xT": np.ascontiguousarray(x[b].T).astype(NPBF16),
            "wqT": np.ascontiguousarray(tq[0 * D_MODEL :][rows].T).astype(NPBF16),
            "wkT": np.ascontiguousarray(tq[1 * D_MODEL :][rows].T).astype(NPBF16),
            "wvT": np.ascontiguousarray(tq[2 * D_MODEL :][rows].T).astype(NPBF16),
            "woT": np.ascontiguousarray(to[:, rows].T).astype(NPBF16),
            "cossinT": cs,
            "sincosT": sc,
        }
        if causal:
            im["maskd"] = maskd
        else:
            im["maskf"] = maskf
        in_maps.append(im)

    do_trace = bool(PROFILE) and _enable_profiling()
    res = run_bass_kernel_spmd(nc, in_maps, list(range(N_CORES)), trace=do_trace)
    LAST_RESULT = res

    parts = [np.asarray(res.results[c]["out"]).astype(np.float32) for c in range(N_CORES)]
    out = np.stack(
        [
            parts[0] + parts[1] + parts[2] + parts[3],
            parts[4] + parts[5] + parts[6] + parts[7],
        ]
    )
    return (out * c2).astype(np.float32)


# revision 17
# speedup vs baseline: 1.0127x; 1.0127x over previous
"""Trainium2 Bass kernel for BitNet multi-head attention (nn_MultiHeadAttention_62294205661880).

Sharding: 8 cores = 2 batches x 4 head-groups (4 heads each).  Each core
computes qkv projection, RoPE, causal attention and a column-parallel slice
of the output projection for its (batch, head-group); the host sums the 4
partial out-projections per batch (the tensor-parallel all-reduce done
host-side, since the contract gathers to host anyway).

BitNet quantization is folded on the host: weights are uploaded as exact
ternary {-1,0,+1} bf16 matrices; scale_qkv^2/sqrt(dh) is folded into the
softmax exp() scale and scale_qkv*scale_out into a final host-side scalar.

Device layout trick: everything is computed transposed.  Q_T/K_T come out of
the projection as [dh, S]; scores are computed as s_T[k, q]; the softmax
denominator sums over the partition (key) dim via an all-ones stationary
matmul (which also replicates the sums across partitions for free); AV
produces out_T[dh, q] which feeds the output projection directly.  No
on-device transposes at all.  Softmax skips the max-subtraction: scores are
bounded (~+-2) because the BitNet weight scale is tiny, so exp() is safe.
"""

import sys
import types

import numpy as np
import ml_dtypes

import concourse.bass as bass
import concourse.mybir as mybir
import concourse.tile as tile
from concourse import bacc
from concourse.bass_utils import run_bass_kernel_spmd

D_MODEL = 2048
N_HEADS = 16
D_HEAD = 128
SEQ = 2048
BATCH = 2
ROPE_BASE = 10000.0

N_CORES = 8
HPC = 4  # heads per core
R_LOCAL = HPC * D_HEAD  # 512 local q (or k, or v) rows per core
MO = D_MODEL // 128  # 16 contraction blocks
NKI = SEQ // 128  # 16 key blocks
NQC = SEQ // 512  # 4 query chunks of 512
NSB = SEQ // 128  # 16 seq blocks (v / proj)

BF16 = mybir.dt.bfloat16
F32 = mybir.dt.float32
NPBF16 = ml_dtypes.bfloat16

LAST_RESULT = None  # BassKernelResults of the most recent run (for test.py)
_PROG_CACHE = {}
PROFILE = False  # test.py sets True to capture an NTFF profile / HW exec time


def _enable_profiling() -> bool:
    """Install the axon NTFF profile hook glue if the image lacks
    ``antenv.axon_hooks`` (boot degrades silently without it), and skip
    the artifact upload (no bucket access in this container)."""
    try:
        from antenv.axon_hooks import get_axon_ntff_profile_hook  # noqa: F401

        ok = get_axon_ntff_profile_hook() is not None
    except ImportError:
        ok = False
        import antenv

        mod = types.ModuleType("antenv.axon_hooks")
        mod._hook = None
        mod.set_axon_ntff_profile_hook = lambda h: setattr(mod, "_hook", h)
        mod.get_axon_ntff_profile_hook = lambda: mod._hook
        sys.modules["antenv.axon_hooks"] = mod
        antenv.axon_hooks = mod
        try:
            from trn_agent_boot.trn_boot import _ntff_profile_via_ctypes

            hook = _ntff_profile_via_ctypes("/opt/axon/libaxon_pjrt.so")
            if hook is not None:
                mod._hook = hook
                ok = True
        except Exception as e:  # profiling is best-effort
            print(f"ntff profile hook install failed: {e}", file=sys.stderr)
    if ok:
        import concourse.bass_utils as _bu

        _bu.upload_artifacts = lambda tmpdir: tmpdir
    return ok


def _build_program(causal: bool, exp_scale: float) -> bass.Bass:
    nc = bacc.Bacc(None)
    S = SEQ

    xT_d = nc.dram_tensor("xT", [D_MODEL, S], BF16, kind="ExternalInput")
    wqT_d = nc.dram_tensor("wqT", [D_MODEL, R_LOCAL], BF16, kind="ExternalInput")
    wkT_d = nc.dram_tensor("wkT", [D_MODEL, R_LOCAL], BF16, kind="ExternalInput")
    wvT_d = nc.dram_tensor("wvT", [D_MODEL, R_LOCAL], BF16, kind="ExternalInput")
    woT_d = nc.dram_tensor("woT", [R_LOCAL, D_MODEL], BF16, kind="ExternalInput")
    # cos rows 0:64, sin rows 64:128
    cs_d = nc.dram_tensor("cossinT", [128, S], BF16, kind="ExternalInput")
    # swapped: sin rows 0:64, cos rows 64:128 (keeps TensorTensor base partitions equal)
    sc_d = nc.dram_tensor("sincosT", [128, S], BF16, kind="ExternalInput")
    if causal:
        # 16 transposed diagonal 128x128 mask blocks, side by side
        maskd_d = nc.dram_tensor("maskd", [128, S], BF16, kind="ExternalInput")
    else:
        maskf_d = nc.dram_tensor("maskf", [S, S], BF16, kind="ExternalInput")
    out_d = nc.dram_tensor("out", [S, D_MODEL], BF16, kind="ExternalOutput")

    xT_v = xT_d[:].rearrange("(mo p) s -> p mo s", p=128)
    wqT_v = wqT_d[:].rearrange("(mo p) r -> p mo r", p=128)
    wkT_v = wkT_d[:].rearrange("(mo p) r -> p mo r", p=128)
    wvT_v = wvT_d[:].rearrange("(mo p) r -> p mo r", p=128)
    woT_v = woT_d[:].rearrange("(h p) o -> p h o", p=128)
    if not causal:
        maskf_v = maskf_d[:].rearrange("(ko p) q -> p ko q", p=128)

    with tile.TileContext(nc) as tc:
        with tc.tile_pool(name="pers", bufs=1) as pers:
            # ---- persistent SBUF tensors (live across both phases) ----
            q_rot = pers.tile([128, HPC, S], BF16, tag="qrot")
            k_rot = pers.tile([128, HPC, S], BF16, tag="krot")
            v_sb = pers.tile([128, NKI, R_LOCAL], BF16, tag="vsb")
            aoT = pers.tile([128, HPC, S], BF16, tag="aoT")
            ones_t = pers.tile([128, 128], BF16, tag="ones")
            warm = pers.tile([128, 1], BF16, tag="warm")
            if causal:
                maskd = pers.tile([128, S], BF16, tag="maskd")
                nc.sync.dma_start(out=maskd[:, :], in_=maskd_d[:, :])
            nc.vector.memset(ones_t[:, :], 1.0)
            # load the exp table set first so no ACT table switch happens
            # mid-kernel (Copy lives in every set).
            nc.scalar.activation(
                warm[:, :], ones_t[:, 0:1], mybir.ActivationFunctionType.Exp
            )

            # ================= phase A: QKV projection + RoPE =================
            with (
                tc.tile_pool(name="xtp", bufs=1) as xtp,
                tc.tile_pool(name="wp", bufs=1) as wp,
                tc.tile_pool(name="raw", bufs=2) as rawp,
                tc.tile_pool(name="tmp", bufs=2) as tmpp,
                tc.tile_pool(name="psA", bufs=2, space="PSUM") as psA,
            ):
                xt = xtp.tile([128, MO, S], BF16, tag="xt")
                wq = wp.tile([128, MO, R_LOCAL], BF16, tag="wq")
                wk = wp.tile([128, MO, R_LOCAL], BF16, tag="wk")
                wv = wp.tile([128, MO, R_LOCAL], BF16, tag="wv")
                cs_t = wp.tile([128, S], BF16, tag="cs")
                sc_t = wp.tile([128, S], BF16, tag="sc")

                nc.sync.dma_start(out=cs_t[:, :], in_=cs_d[:, :])
                nc.sync.dma_start(out=sc_t[:, :], in_=sc_d[:, :])
                for mo in range(MO):
                    nc.sync.dma_start(out=wq[:, mo, :], in_=wqT_v[:, mo, :])
                    nc.sync.dma_start(out=wk[:, mo, :], in_=wkT_v[:, mo, :])
                    if mo < 4:
                        nc.sync.dma_start(
                            out=xt[:, mo, 0:1024], in_=xT_v[:, mo, 0:1024]
                        )
                        nc.sync.dma_start(
                            out=xt[:, mo, 1024:2048], in_=xT_v[:, mo, 1024:2048]
                        )
                    else:
                        nc.sync.dma_start(out=xt[:, mo, :], in_=xT_v[:, mo, :])
                for mo in range(MO):
                    nc.sync.dma_start(out=wv[:, mo, :], in_=wvT_v[:, mo, :])

                def rope(dst, raw):
                    """NeoX rotary: rows 0:64 = t*c - b*s ; rows 64:128 = t*s + b*c."""
                    ta = tmpp.tile([64, S], BF16, tag="tmp")
                    tb = tmpp.tile([64, S], BF16, tag="tmp")
                    nc.vector.tensor_mul(ta[:, :], raw[0:64, :], cs_t[0:64, :])
                    nc.vector.tensor_mul(tb[:, :], raw[64:128, :], cs_t[64:128, :])
                    nc.vector.tensor_sub(dst[0:64, :], ta[:, :], tb[:, :])
                    tc2 = tmpp.tile([64, S], BF16, tag="tmp")
                    td = tmpp.tile([64, S], BF16, tag="tmp")
                    nc.vector.tensor_mul(tc2[:, :], raw[0:64, :], sc_t[0:64, :])
                    nc.vector.tensor_mul(td[:, :], raw[64:128, :], sc_t[64:128, :])
                    nc.vector.tensor_add(dst[64:128, :], tc2[:, :], td[:, :])

                # head 0 q/k with the m-loop OUTER so the matmuls consume
                # xt m-blocks as the DMAs land (startup overlap).
                qp0 = psA.tile([128, S], F32, tag="psA")
                kp0 = psA.tile([128, S], F32, tag="psA")
                for m in range(MO):
                    for c4 in range(4):
                        nc.tensor.matmul(
                            qp0[:, c4 * 512 : (c4 + 1) * 512],
                            wq[:, m, 0:128],
                            xt[:, m, c4 * 512 : (c4 + 1) * 512],
                            start=(m == 0),
                            stop=(m == MO - 1),
                        )
                        nc.tensor.matmul(
                            kp0[:, c4 * 512 : (c4 + 1) * 512],
                            wk[:, m, 0:128],
                            xt[:, m, c4 * 512 : (c4 + 1) * 512],
                            start=(m == 0),
                            stop=(m == MO - 1),
                        )
                q_raw = rawp.tile([128, S], BF16, tag="raw")
                nc.scalar.copy(q_raw[:, :], qp0[:, :])
                rope(q_rot[:, 0, :], q_raw)
                k_raw = rawp.tile([128, S], BF16, tag="raw")
                nc.scalar.copy(k_raw[:, :], kp0[:, :])
                rope(k_rot[:, 0, :], k_raw)

                def project(dst_raw, w_sb, h):
                    """q/k head projection -> bf16 raw [128, S] (xt resident)."""
                    ps = psA.tile([128, S], F32, tag="psA")
                    for c4 in range(4):
                        for m in range(MO):
                            nc.tensor.matmul(
                                ps[:, c4 * 512 : (c4 + 1) * 512],
                                w_sb[:, m, h * 128 : (h + 1) * 128],
                                xt[:, m, c4 * 512 : (c4 + 1) * 512],
                                start=(m == 0),
                                stop=(m == MO - 1),
                            )
                    nc.scalar.copy(dst_raw[:, :], ps[:, :])

                for h in range(1, HPC):
                    q_raw = rawp.tile([128, S], BF16, tag="raw")
                    project(q_raw, wq, h)
                    rope(q_rot[:, h, :], q_raw)
                    k_raw = rawp.tile([128, S], BF16, tag="raw")
                    project(k_raw, wk, h)
                    rope(k_rot[:, h, :], k_raw)

                # V projection (natural layout [s, r]); 4 seq blocks per psum
                for sb4 in range(NSB // 4):
                    ps = psA.tile([128, S], F32, tag="psA")
                    for part in range(4):
                        sb = sb4 * 4 + part
                        for m in range(MO):
                            nc.tensor.matmul(
                                ps[:, part * 512 : part * 512 + 512],
                                xt[:, m, sb * 128 : (sb + 1) * 128],
                                wv[:, m, :],
                                start=(m == 0),
                                stop=(m == MO - 1),
                            )
                    nc.scalar.copy(v_sb[:, sb4 * 4 : sb4 * 4 + 4, :], ps[:, :])

            # ================= phase B: attention + out-projection =============
            with (
                tc.tile_pool(name="wop", bufs=1) as wop,
                tc.tile_pool(name="pp", bufs=8) as ppp,
                tc.tile_pool(name="rcp", bufs=3) as rcp,
                tc.tile_pool(name="osb", bufs=4) as osbp,
                tc.tile_pool(name="mblk", bufs=4) as mblkp,
                tc.tile_pool(name="sp", bufs=4, space="PSUM") as spp,
                tc.tile_pool(name="acc", bufs=2, space="PSUM") as accp,
            ):
                wo = wop.tile([128, HPC, D_MODEL], BF16, tag="wo")
                for oc in range(D_MODEL // 512):
                    nc.sync.dma_start(
                        out=wo[:, :, oc * 512 : (oc + 1) * 512],
                        in_=woT_v[:, :, oc * 512 : (oc + 1) * 512],
                    )

                evict_flip = [0]

                for qc in range(NQC):
                    q_lo = qc * 512
                    nki_here = (4 * qc + 4) if causal else NKI
                    for h in range(HPC):
                        sav = accp.tile([128, 1024], F32, tag="acc")
                        sums = sav[:, 0:512]
                        avp = sav[:, 512:1024]
                        for ki in range(nki_here):
                            diag = causal and ki >= 4 * qc
                            q0 = 128 * (ki - 4 * qc) if diag else 0
                            spb = spp.tile([128, 512], F32, tag="sp")
                            pp = ppp.tile([128, 512], BF16, tag="pp")
                            nc.tensor.matmul(
                                spb[:, q0:512],
                                k_rot[:, h, ki * 128 : (ki + 1) * 128],
                                q_rot[:, h, q_lo + q0 : q_lo + 512],
                                start=True,
                                stop=True,
                            )
                            nc.scalar.activation(
                                pp[:, q0:512],
                                spb[:, q0:512],
                                mybir.ActivationFunctionType.Exp,
                                scale=float(exp_scale),
                            )
                            if causal:
                                if diag:
                                    nc.vector.tensor_mul(
                                        pp[:, q0 : q0 + 128],
                                        pp[:, q0 : q0 + 128],
                                        maskd[:, ki * 128 : (ki + 1) * 128],
                                    )
                            else:
                                mb = mblkp.tile([128, 512], BF16, tag="mblk")
                                nc.sync.dma_start(
                                    out=mb[:, :],
                                    in_=maskf_v[:, ki, q_lo : q_lo + 512],
                                )
                                nc.vector.tensor_mul(
                                    pp[:, 0:512], pp[:, 0:512], mb[:, :]
                                )
                            nc.tensor.matmul(
                                sums[:, q0:512],
                                ones_t[:, :],
                                pp[:, q0:512],
                                start=(ki == 0),
                                stop=(ki == nki_here - 1),
                            )
                            nc.tensor.matmul(
                                avp[:, q0:512],
                                v_sb[:, ki, h * 128 : (h + 1) * 128],
                                pp[:, q0:512],
                                start=(ki == 0),
                                stop=(ki == nki_here - 1),
                            )
                        rc = rcp.tile([128, 512], F32, tag="rc")
                        nc.vector.reciprocal_approx_fast(rc[:, :], sums[:, :])
                        nc.vector.tensor_mul(
                            aoT[:, h, q_lo : q_lo + 512], avp[:, :], rc[:, :]
                        )

                    # out-projection for this query chunk (4 seq blocks);
                    # h outer over oc pairs so each aoT stationary load
                    # serves two matmuls
                    for sb in range(4 * qc, 4 * qc + 4):
                        for oc2 in range(2):
                            op2 = accp.tile([128, 1024], F32, tag="acc")
                            for h in range(HPC):
                                lhsT = aoT[:, h, sb * 128 : (sb + 1) * 128]
                                nc.tensor.matmul(
                                    op2[:, 0:512],
                                    lhsT,
                                    wo[:, h, (2 * oc2) * 512 : (2 * oc2 + 1) * 512],
                                    start=(h == 0),
                                    stop=(h == HPC - 1),
                                )
                                nc.tensor.matmul(
                                    op2[:, 512:1024],
                                    lhsT,
                                    wo[:, h, (2 * oc2 + 1) * 512 : (2 * oc2 + 2) * 512],
                                    start=(h == 0),
                                    stop=(h == HPC - 1),
                                )
                            ob = osbp.tile([128, 1024], BF16, tag="osb")
                            if evict_flip[0] % 2 == 0:
                                nc.scalar.copy(ob[:, :], op2[:, :])
                            else:
                                nc.vector.tensor_copy(ob[:, :], op2[:, :])
                            evict_flip[0] += 1
                            nc.sync.dma_start(
                                out=out_d[
                                    sb * 128 : (sb + 1) * 128,
                                    oc2 * 1024 : (oc2 + 1) * 1024,
                                ],
                                in_=ob[:, :],
                            )

    nc.finalize()
    return nc


def _bit_quantize_ternary(w: np.ndarray):
    """Returns (ternary {-1,0,1} float32 matrix, scale) matching the reference."""
    scale = np.maximum(np.mean(np.abs(w.astype(np.float32))), np.float32(1e-5))
    t = np.clip(np.round(w.astype(np.float32) / scale), -1.0, 1.0).astype(np.float32)
    return t, float(scale)


def _host_tables():
    """cos/sin stacked [128, S]: rows 0:64 cos, rows 64:128 sin."""
    inv_freq = 1.0 / (ROPE_BASE ** (np.arange(0, D_HEAD, 2, dtype=np.float32) / D_HEAD))
    pos = np.arange(SEQ, dtype=np.float32)
    ang = pos[:, None] * inv_freq[None, :]  # [S, 64]
    cs = np.empty((128, SEQ), dtype=NPBF16)
    cs[0:64] = np.ascontiguousarray(np.cos(ang).T).astype(NPBF16)
    cs[64:128] = np.ascontiguousarray(np.sin(ang).T).astype(NPBF16)
    sc = np.empty((128, SEQ), dtype=NPBF16)
    sc[0:64] = cs[64:128]
    sc[64:128] = cs[0:64]
    return cs, sc


def kernel(x, w_qkv, w_out, mask):
    global LAST_RESULT
    x = np.asarray(x, dtype=np.float32)
    w_qkv = np.asarray(w_qkv, dtype=np.float32)
    w_out = np.asarray(w_out, dtype=np.float32)
    mask = np.asarray(mask)

    tq, sq = _bit_quantize_ternary(w_qkv)
    to, so = _bit_quantize_ternary(w_out)
    exp_scale = (sq * sq) / float(np.sqrt(D_HEAD))
    c2 = np.float32(sq * so)

    m2 = (mask.reshape(SEQ, SEQ) != 0).astype(np.float32)
    causal = bool(np.array_equal(m2, np.tril(np.ones((SEQ, SEQ), np.float32))))

    cs, sc = _host_tables()
    if causal:
        maskd = np.empty((128, SEQ), dtype=NPBF16)
        for ki in range(NKI):
            blk = m2[ki * 128 : (ki + 1) * 128, ki * 128 : (ki + 1) * 128]  # [q, k]
            maskd[:, ki * 128 : (ki + 1) * 128] = np.ascontiguousarray(blk.T).astype(
                NPBF16
            )
    else:
        maskf = np.ascontiguousarray(m2.T).astype(NPBF16)  # [kk, qq]

    key = (causal, float(exp_scale))
    if key not in _PROG_CACHE:
        _PROG_CACHE[key] = _build_program(causal, float(exp_scale))
    nc = _PROG_CACHE[key]

    in_maps = []
    for c in range(N_CORES):
        b, g = divmod(c, 4)
        rows = slice(R_LOCAL * g, R_LOCAL * (g + 1))
        im = {
            "xT": np.ascontiguousarray(x[b].T).astype(NPBF16),
            "wqT": np.ascontiguousarray(tq[0 * D_MODEL :][rows].T).astype(NPBF16),
            "wkT": np.ascontiguousarray(tq[1 * D_MODEL :][rows].T).astype(NPBF16),
            "wvT": np.ascontiguousarray(tq[2 * D_MODEL :][rows].T).astype(NPBF16),
            "woT": np.ascontiguousarray(to[:, rows].T).astype(NPBF16),
            "cossinT": cs,
            "sincosT": sc,
        }
        if causal:
            im["maskd"] = maskd
        else:
            im["maskf"] = maskf
        in_maps.append(im)

    do_trace = bool(PROFILE) and _enable_profiling()
    res = run_bass_kernel_spmd(nc, in_maps, list(range(N_CORES)), trace=do_trace)
    LAST_RESULT = res

    parts = [np.asarray(res.results[c]["out"]).astype(np.float32) for c in range(N_CORES)]
    out = np.stack(
        [
            parts[0] + parts[1] + parts[2] + parts[3],
            parts[4] + parts[5] + parts[6] + parts[7],
        ]
    )
    return (out * c2).astype(np.float32)
